# revision 15
# baseline (speedup 1.0000x reference)
"""Causal self-attention (GQA + QK-RMSNorm + RoPE + q_gain) on 8 Trainium2 cores.

Sharding: 8 cores = 2 (batch) x 4 (KV head group).  Core c handles batch
c//4 and KV head g=c%4, i.e. Q heads 4g..4g+3.  Each core computes its
heads' attention and a partial output projection (its 512 columns of the
attention output against the matching 512 rows of Wproj^T); the host sums
the 4 partials per batch.

All shapes are hardcoded for B=2, S=2048, D=2048, H=16, KVH=4, HD=128.
Matmuls run as float32r (full PE rate at N>=256, ~tf32 precision), fp32
storage everywhere.
"""

import numpy as np

B, S, D = 2, 2048, 2048
H, KVH = 16, 4
HD = 128  # head dim
G = H // KVH  # q heads per kv group = 4
NCORES = 8
ROPE_BASE = 10000.0
EPS = 1e-6

P = 128          # partitions
SL = 512         # token slice for phase 1 / proj
NSL = S // SL    # 4
DK = D // P      # 16 contraction subtiles
NMEGA = 2        # attention processes sq in mega-slices of 1024
MEGA = 1024

_CACHE = {}


def _build_program():
    """Build + compile the (single, SPMD) Bass program. Returns nc."""
    from contextlib import ExitStack

    import concourse.bass as bass
    import concourse.tile as tile
    from concourse import bacc, mybir
    from concourse.masks import make_identity

    f32 = mybir.dt.float32
    f32r = mybir.dt.float32r
    AF = mybir.ActivationFunctionType
    OP = mybir.AluOpType

    nc = bacc.Bacc("TRN2", target_bir_lowering=False)

    xT_d = nc.dram_tensor("xT", [D, S], f32, kind="ExternalInput").ap()
    wqT_d = nc.dram_tensor("wqT", [D, G * HD], f32, kind="ExternalInput").ap()
    wkT_d = nc.dram_tensor("wkT", [D, HD], f32, kind="ExternalInput").ap()
    wvT_d = nc.dram_tensor("wvT", [D, HD], f32, kind="ExternalInput").ap()
    wpT_d = nc.dram_tensor("wpT", [G * HD, D], f32, kind="ExternalInput").ap()
    cosT_d = nc.dram_tensor("cosT", [HD, S], f32, kind="ExternalInput").ap()
    sinT_d = nc.dram_tensor("sinT", [HD, S], f32, kind="ExternalInput").ap()
    jT_d = nc.dram_tensor("jT", [HD, HD], f32, kind="ExternalInput").ap()
    qgain_d = nc.dram_tensor("qgain", [1, G], f32, kind="ExternalInput").ap()
    y_d = nc.dram_tensor("y", [S, D], f32, kind="ExternalOutput").ap()

    with tile.TileContext(nc) as tc, ExitStack() as top:
        res = top.enter_context(tc.tile_pool(name="resident", bufs=1))

        # ---- small constants ----
        ones_f = res.tile([P, 1], f32)
        nc.vector.memset(ones_f[:], 1.0)
        ones = res.tile([P, 1], f32r)
        nc.vector.tensor_copy(ones[:], ones_f[:])
        ident = res.tile([P, P], f32)
        make_identity(nc, ident[:])
        eps_t = res.tile([1, 1], f32)
        nc.vector.memset(eps_t[:], EPS)
        qgain = res.tile([1, G], f32)
        nc.sync.dma_start(qgain[:], qgain_d[:])
        jT = res.tile([HD, HD], f32r)
        nc.gpsimd.dma_start(jT[:], jT_d[:])

        # ---- resident Q^T/K^T/V ----
        qT = [res.tile([P, S], f32r, tag=f"qT{h}", name=f"qT{h}") for h in range(G)]
        kT = res.tile([P, S], f32r)
        v_sb = res.tile([P, S // P, HD], f32r)  # V natural, [s_inner, s_tile, hd]

        # ================= PHASE 1: QKV + RMSNorm + RoPE =================
        with ExitStack() as ph1:
            wq = ph1.enter_context(tc.tile_pool(name="wq", bufs=1))
            xtp = ph1.enter_context(tc.tile_pool(name="xt", bufs=18))
            tmp = ph1.enter_context(tc.tile_pool(name="p1tmp", bufs=3))
            rowp = ph1.enter_context(tc.tile_pool(name="p1row", bufs=3))
            csp = ph1.enter_context(tc.tile_pool(name="cs", bufs=1))
            ps1 = ph1.enter_context(tc.tile_pool(name="ps1", bufs=2, space="PSUM"))
            psj = ph1.enter_context(tc.tile_pool(name="psj", bufs=2, space="PSUM"))
            pssq = ph1.enter_context(tc.tile_pool(name="pssq", bufs=2, space="PSUM"))
            pstr = ph1.enter_context(tc.tile_pool(name="pstr", bufs=2, space="PSUM"))

            wq_sb = wq.tile([P, DK, G * HD], f32r)
            nc.gpsimd.dma_start(wq_sb[:], wqT_d.rearrange("(o p) m -> p o m", p=P))
            wk_sb = wq.tile([P, DK, HD], f32r)
            nc.gpsimd.dma_start(wk_sb[:], wkT_d.rearrange("(o p) m -> p o m", p=P))
            wv_sb = wq.tile([P, DK, HD], f32r)
            nc.gpsimd.dma_start(wv_sb[:], wvT_d.rearrange("(o p) m -> p o m", p=P))
            cos_sb = csp.tile([HD, S], f32)
            nc.sync.dma_start(cos_sb[:], cosT_d[:])
            sin_sb = csp.tile([HD, S], f32)
            nc.sync.dma_start(sin_sb[:], sinT_d[:])

            xT3 = xT_d.rearrange("(o p) s -> p o s", p=P)

            def norm_rope(src_ps, dst, js, gain_ap):
                """RMS-normalize (+optional gain) and RoPE a [128, SL] head block.

                src_ps: PSUM tile [P, SL] holding raw projection output.
                dst: SBUF AP [P, SL] (slice of resident q/k tile).
                gain_ap: [1,1] AP with gain/sqrt(HD) premultiplied, or None.
                """
                sq = tmp.tile([P, SL], f32r, tag="sq")
                nc.scalar.square(sq[:], src_ps[:])
                ssq = pssq.tile([1, SL], f32, tag="ssq")
                nc.tensor.matmul(ssq[:], ones[:], sq[:], start=True, stop=True)
                # f = 1/sqrt(ssq/HD + eps)  (x gain)
                frow = rowp.tile([1, SL], f32, tag="frow")
                nc.scalar.activation(frow[:], ssq[:], AF.Sqrt,
                                     bias=eps_t[:], scale=1.0 / HD)
                nc.vector.reciprocal(frow[:], frow[:])
                if gain_ap is not None:
                    nc.vector.tensor_scalar_mul(frow[:], frow[:], gain_ap)
                fb = tmp.tile([P, SL], f32, tag="fb")
                nc.gpsimd.partition_broadcast(fb[:], frow[:])
                qn = tmp.tile([P, SL], f32r, tag="qn")
                nc.vector.tensor_mul(qn[:], src_ps[:], fb[:])
                # rope: rot(q) = qn*cosf + (J @ qn)*sinf, J = [[0,I],[-I,0]]
                qj = psj.tile([P, SL], f32, tag="qj")
                nc.tensor.matmul(qj[:], jT[:], qn[:], start=True, stop=True)
                c = cos_sb[:, js * SL:(js + 1) * SL]
                s = sin_sb[:, js * SL:(js + 1) * SL]
                t1 = tmp.tile([P, SL], f32, tag="t1")
                t2 = tmp.tile([P, SL], f32, tag="t2")
                nc.vector.tensor_mul(t1[:], qn[:], c)
                nc.vector.tensor_mul(t2[:], qj[:], s)
                nc.vector.tensor_add(dst, t1[:], t2[:])

            for js in range(NSL):
                xts = [xtp.tile([P, SL], f32r, tag="xt", name=f"xt_{js}_{dk}")
                       for dk in range(DK)]
                for dk in range(DK):
                    nc.gpsimd.dma_start(xts[dk][:], xT3[:, dk, js * SL:(js + 1) * SL])
                # Q heads
                for h in range(G):
                    q_ps = ps1.tile([P, SL], f32, tag="qkv")
                    for dk in range(DK):
                        nc.tensor.matmul(
                            q_ps[:], wq_sb[:, dk, h * HD:(h + 1) * HD],
                            xts[dk][:], start=(dk == 0), stop=(dk == DK - 1))
                    norm_rope(q_ps, qT[h][:, js * SL:(js + 1) * SL], js,
                              qgain[0:1, h:h + 1])
                # K
                k_ps = ps1.tile([P, SL], f32, tag="qkv")
                for dk in range(DK):
                    nc.tensor.matmul(k_ps[:], wk_sb[:, dk, :], xts[dk][:],
                                     start=(dk == 0), stop=(dk == DK - 1))
                norm_rope(k_ps, kT[:, js * SL:(js + 1) * SL], js, None)
                # V: compute V^T then PE-transpose to natural layout
                v_ps = ps1.tile([P, SL], f32, tag="qkv")
                for dk in range(DK):
                    nc.tensor.matmul(v_ps[:], wv_sb[:, dk, :], xts[dk][:],
                                     start=(dk == 0), stop=(dk == DK - 1))
                vt = tmp.tile([P, SL], f32, tag="vt")
                nc.vector.tensor_copy(vt[:], v_ps[:])
                for t in range(SL // P):
                    vtr_ps = pstr.tile([P, P], f32, tag="vtr")
                    nc.tensor.transpose(vtr_ps[:], vt[:, t * P:(t + 1) * P], ident[:])
                    nc.scalar.copy(v_sb[:, js * (SL // P) + t, :], vtr_ps[:])

        # ================= PHASE 2: attention + proj =====================
        with ExitStack() as ph2:
            wpp = ph2.enter_context(tc.tile_pool(name="wp", bufs=1))
            otp = ph2.enter_context(tc.tile_pool(name="oT", bufs=1))
            ptp = ph2.enter_context(tc.tile_pool(name="pt", bufs=3))
            rowp2 = ph2.enter_context(tc.tile_pool(name="p2row", bufs=3))
            bb = ph2.enter_context(tc.tile_pool(name="p2b", bufs=2))

            wp_sb = wpp.tile([P, G, D], f32r)
            nc.gpsimd.dma_start(wp_sb[:], wpT_d.rearrange("(o p) m -> p o m", p=P))
            oT = [otp.tile([P, S], f32r, tag=f"oT{h}", name=f"oT{h}") for h in range(G)]

            with ExitStack() as attn:
                pssc = attn.enter_context(
                    tc.tile_pool(name="pssc", bufs=2, space="PSUM"))
                pso = attn.enter_context(
                    tc.tile_pool(name="pso", bufs=1, space="PSUM"))
                psrs = attn.enter_context(
                    tc.tile_pool(name="psrs", bufs=1, space="PSUM"))

                for h in range(G):
                    for j2 in range(NMEGA):
                        o_ps = pso.tile([P, MEGA], f32, tag="o")
                        rs_ps = psrs.tile([1, MEGA], f32, tag="rs")
                        ilast = 8 * j2 + 7       # last key tile index
                        i0max = 8 * j2 + 3       # last valid i for sub-slice 0
                        for i in range(ilast + 1):
                            sub0 = i <= i0max
                            sc = pssc.tile([P, MEGA], f32, tag="sc")
                            pt = ptp.tile([P, MEGA], f32r, tag="pt")
                            lhs_k = kT[:, i * P:(i + 1) * P]
                            for sub in (0, 1):
                                if sub == 0 and not sub0:
                                    continue
                                js = 2 * j2 + sub
                                nc.tensor.matmul(
                                    sc[:, sub * SL:(sub + 1) * SL], lhs_k,
                                    qT[h][:, js * SL:(js + 1) * SL],
                                    start=True, stop=True)
                            lo = 0 if sub0 else SL
                            nc.scalar.activation(pt[:, lo:MEGA], sc[:, lo:MEGA],
                                                 AF.Exp)
                            for sub in (0, 1):
                                js = 2 * j2 + sub
                                delta = i - 4 * js
                                if 0 <= delta <= 3:
                                    nc.gpsimd.affine_select(
                                        out=pt[:, sub * SL:(sub + 1) * SL],
                                        in_=pt[:, sub * SL:(sub + 1) * SL],
                                        compare_op=OP.is_ge, fill=0.0,
                                        base=-P * delta, pattern=[[1, SL]],
                                        channel_multiplier=-1)
                            for sub in (0, 1):
                                if sub == 0 and not sub0:
                                    continue
                                pts = pt[:, sub * SL:(sub + 1) * SL]
                                stop = i == (i0max if sub == 0 else ilast)
                                nc.tensor.matmul(
                                    rs_ps[:, sub * SL:(sub + 1) * SL],
                                    ones[:], pts, start=(i == 0), stop=stop)
                                nc.tensor.matmul(
                                    o_ps[:, sub * SL:(sub + 1) * SL],
                                    v_sb[:, i, :], pts,
                                    start=(i == 0), stop=stop)
                        # normalize: oT = o_ps / rowsum
                        rrow = rowp2.tile([1, MEGA], f32, tag="rrow")
                        nc.vector.reciprocal(rrow[:], rs_ps[:])
                        rb = bb.tile([P, MEGA], f32, tag="rb")
                        nc.gpsimd.partition_broadcast(rb[:], rrow[:])
                        nc.vector.tensor_mul(
                            oT[h][:, j2 * MEGA:(j2 + 1) * MEGA], o_ps[:], rb[:])

            # ---- output projection: y[s,o] partial ----
            with ExitStack() as proj:
                psy = proj.enter_context(
                    tc.tile_pool(name="psy", bufs=4, space="PSUM"))
                yout = proj.enter_context(tc.tile_pool(name="yout", bufs=4))
                for st in range(S // P):
                    for os_ in range(D // SL):
                        y_ps = psy.tile([P, SL], f32, tag="y")
                        for h in range(G):
                            nc.tensor.matmul(
                                y_ps[:], oT[h][:, st * P:(st + 1) * P],
                                wp_sb[:, h, os_ * SL:(os_ + 1) * SL],
                                start=(h == 0), stop=(h == G - 1))
                        y_sb = yout.tile([P, SL], f32, tag="ysb")
                        nc.scalar.copy(y_sb[:], y_ps[:])
                        nc.sync.dma_start(
                            y_d[st * P:(st + 1) * P, os_ * SL:(os_ + 1) * SL],
                            y_sb[:])

    nc.compile()
    return nc


def _rope_tables():
    """cos/sin tables in [HD, S] layout (half-tables stacked twice), plus J^T."""
    inv_freq = 1.0 / (ROPE_BASE ** (np.arange(0, HD, 2, dtype=np.float32) / HD))
    freqs = np.outer(np.arange(S, dtype=np.float32), inv_freq)  # [S, half]
    c = np.cos(freqs).T.astype(np.float32)  # [half, S]
    s = np.sin(freqs).T.astype(np.float32)
    cosf = np.concatenate([c, c], axis=0).copy()  # [HD, S]
    sinf = np.concatenate([s, s], axis=0).copy()
    half = HD // 2
    jT = np.zeros((HD, HD), np.float32)
    jT[np.arange(half) + half, np.arange(half)] = 1.0   # (Jq)[j] = q[j+64], j<64
    jT[np.arange(half), np.arange(half) + half] = -1.0  # (Jq)[j+64] = -q[j]
    return cosf, sinf, jT


def make_in_maps(x, Wq, Wk, Wv, Wproj, q_gain):
    """Host-side shard prep: per-core input dicts."""
    cosT, sinT, jT = _rope_tables()
    xT = np.ascontiguousarray(np.transpose(np.asarray(x, np.float32), (0, 2, 1)))
    Wq = np.asarray(Wq, np.float32)
    Wk = np.asarray(Wk, np.float32)
    Wv = np.asarray(Wv, np.float32)
    WpT = np.ascontiguousarray(np.asarray(Wproj, np.float32).T)  # [in, out]
    q_gain = np.asarray(q_gain, np.float32)

    in_maps = []
    for c in range(NCORES):
        b, g = divmod(c, KVH)
        sl_q = slice(g * G * HD, (g + 1) * G * HD)
        sl_kv = slice(g * HD, (g + 1) * HD)
        in_maps.append({
            "xT": xT[b],
            "wqT": np.ascontiguousarray(Wq[sl_q, :].T),
            "wkT": np.ascontiguousarray(Wk[sl_kv, :].T),
            "wvT": np.ascontiguousarray(Wv[sl_kv, :].T),
            "wpT": np.ascontiguousarray(WpT[sl_q, :]),
            "cosT": cosT,
            "sinT": sinT,
            "jT": jT,
            "qgain": (q_gain[g * G:(g + 1) * G] / np.sqrt(HD))
            .reshape(1, G).astype(np.float32),
        })
    return in_maps


def kernel(x, Wq, Wk, Wv, Wproj, q_gain):
    from concourse.bass_utils import run_bass_kernel_spmd

    if "nc" not in _CACHE:
        _CACHE["nc"] = _build_program()
    nc = _CACHE["nc"]

    in_maps = make_in_maps(x, Wq, Wk, Wv, Wproj, q_gain)
    res = run_bass_kernel_spmd(nc, in_maps, core_ids=list(range(NCORES)))
    _CACHE["last_results"] = res

    y = np.zeros((B, S, D), dtype=np.float32)
    for c in range(NCORES):
        y[c // KVH] += res.results[c]["y"]
    return y
